# revision 10
# baseline (speedup 1.0000x reference)
"""Correlation1D Trainium2 Bass kernel.

out[b, d, h, w] = (1/C) * sum_c in1[b, c, h, w] * in2pad[b, c, h, w + d]
  B=8, C=256, H=96, W=192, PAD=40, D=81 displacement channels.

Strategy (data-parallel over batch, 1 sample per NeuronCore):
  For each h row and each w-chunk of 96, compute a Gram band
      G[w, w'] = sum_c in1[c, w] * in2pad[c, w']      (PE matmuls, k=c)
  for w' in [ck*96, ck*96 + 176).  The needed outputs are the 81
  diagonals O[d, w] = G[w, w + d].  Diagonal extraction is a
  per-partition-offset move: partition w needs band columns [w, w+81).
  GPSIMD's local_scatter supports per-partition independent indices
  (dst[p, idx[p,i]] = data[p,i], -1 skips), so a static int16 index
  tile (fed as an extra kernel input) extracts all diagonals on-chip —
  no DRAM scratch round-trip.  HBM traffic is just inputs + output
  (43.7 MB/core).  A PE transpose then turns T[w, d] tiles into
  O[d, w] tiles written out in the final [d, h, w] layout.

  Inputs load as fp32 over HWDGE (keeping descriptor-gen off GPSIMD,
  which local_scatter saturates) and are cast fp32->fp16 on the DVE.
  Matmuls run in fp16 (full PE rate at any moving width, so the rhs is
  just the 176-wide band — no 256-pad needed).  fp16 inputs keep
  ~2^-11 element error; the fp16 band (pre-scaled by 1/C) adds ~5e-4.
"""

import os

import numpy as np

import concourse.bass as bass
import concourse.tile as tile
from concourse import bacc, mybir
from concourse.bass_utils import run_bass_kernel_spmd

# Problem constants (hardcoded per harness contract)
B = 8
C = 256
H = 96
W = 192
PAD = 40
D = 2 * PAD + 1  # 81
W2 = W + 2 * PAD  # 272 padded width
CH = 2  # c is split into CH partition-halves of 128
CP = C // CH  # 128
CHUNK = 96  # w-chunk (Gram output partition dim)
NCK = W // CHUNK  # 2
BANDW = CHUNK + D - 1  # 176  (w' window width per chunk)
DE = D + 1  # 82: even-sized diagonal slot per h row (local_scatter needs %2)

# Tunables (env-overridable for experiments)
HB = int(os.environ.get("CORR_HB", "4"))  # h rows per batch
NB = H // HB
BAND_DT_S = os.environ.get("CORR_BAND_DT", "fp16")  # fp16 | bf16
MM_DT_S = os.environ.get("CORR_MM", "fp16")  # fp16 | bf16
IN_BUFS = int(os.environ.get("CORR_IN_BUFS", "3"))
G_BUFS = int(os.environ.get("CORR_G_BUFS", "4"))

_DT = {
    "fp32": mybir.dt.float32,
    "fp16": mybir.dt.float16,
    "bf16": mybir.dt.bfloat16,
}


def make_diag_idx() -> np.ndarray:
    """Static local_scatter indices: idx[w, hl*BANDW + j] = hl*DE + (j - w)
    when 0 <= j - w < D, else -1 (skipped)."""
    idx = np.full((CHUNK, HB * BANDW), -1, dtype=np.int16)
    w = np.arange(CHUNK)[:, None]
    j = np.arange(BANDW)[None, :]
    d = j - w  # [CHUNK, BANDW]
    valid = (d >= 0) & (d < D)
    for hl in range(HB):
        blk = np.where(valid, hl * DE + d, -1).astype(np.int16)
        idx[:, hl * BANDW : (hl + 1) * BANDW] = blk
    return idx


def _build(reps=1):
    band_dt = _DT[BAND_DT_S]
    mm_dt = _DT[MM_DT_S]
    f32 = mybir.dt.float32
    i16 = mybir.dt.int16

    nc = bacc.Bacc("TRN2")

    in1 = nc.dram_tensor("input1", [C, H, W], f32, kind="ExternalInput")
    in2 = nc.dram_tensor("input2", [C, H, W], f32, kind="ExternalInput")
    didx = nc.dram_tensor("didx", [CHUNK, HB * BANDW], i16, kind="ExternalInput")
    out = nc.dram_tensor("out", [D, H, W], f32, kind="ExternalOutput")

    # [c, h, w] -> [p, a, h*w] so each input load is one 3-dim DMA
    in1_r = in1.ap().rearrange("(a p) h w -> p a (h w)", p=CP)
    in2_r = in2.ap().rearrange("(a p) h w -> p a (h w)", p=CP)
    out_ap = out.ap()

    with tile.TileContext(nc) as tc:
        with (
            tc.tile_pool(name="singles", bufs=1) as singles,
            tc.tile_pool(name="loads", bufs=IN_BUFS) as loads,
            tc.tile_pool(name="casts", bufs=2) as casts,
            tc.tile_pool(name="bands", bufs=2) as bands,
            tc.tile_pool(name="gats", bufs=2) as gats,
            tc.tile_pool(name="outs", bufs=2) as outs,
            tc.tile_pool(name="psg", bufs=G_BUFS, space="PSUM") as psg,
            tc.tile_pool(name="pso", bufs=2, space="PSUM") as pso,
        ):
            # identity for PE transposes
            ident = singles.tile([CHUNK, CHUNK], band_dt)
            from concourse.masks import make_identity

            make_identity(nc, ident[:])

            # static per-partition diagonal indices, loaded once
            idx_t = singles.tile([CHUNK, HB * BANDW], i16)
            nc.sync.dma_start(out=idx_t[:], in_=didx.ap())

            for _rep in range(reps):
              for ib in range(NB):
                h0 = ib * HB

                in1_t = loads.tile([CP, CH, HB * W], f32)
                nc.sync.dma_start(
                    out=in1_t[:],
                    in_=in1_r[:, :, h0 * W : (h0 + HB) * W],
                )
                in2_t = loads.tile([CP, CH, HB * W], f32)
                nc.sync.dma_start(
                    out=in2_t[:],
                    in_=in2_r[:, :, h0 * W : (h0 + HB) * W],
                )

                # fp32 -> mm_dt casts on DVE; in2 goes into the interior of
                # a zero-padded 272-wide tile (band windows reach the pads)
                in1_c = casts.tile([CP, CH, HB, W], mm_dt)
                nc.vector.tensor_copy(
                    out=in1_c[:].rearrange("p a h w -> p a (h w)"), in_=in1_t[:]
                )
                in2_c = casts.tile([CP, CH, HB, W2], mm_dt)
                nc.vector.memset(in2_c[:, :, :, 0:PAD], 0.0)
                nc.vector.memset(in2_c[:, :, :, PAD + W : W2], 0.0)
                nc.vector.tensor_copy(
                    out=in2_c[:, :, :, PAD : PAD + W],
                    in_=in2_t[:].rearrange("p a (h w) -> p a h w", h=HB),
                )

                band_ts = [
                    bands.tile(
                        [CHUNK, HB, BANDW], band_dt,
                        name=f"band{ck}_{_rep}_{ib}", tag=f"band{ck}",
                    )
                    for ck in range(NCK)
                ]

                for hl in range(HB):
                    for ck in range(NCK):
                        g = psg.tile([CHUNK, BANDW], f32)
                        for a in range(CH):
                            nc.tensor.matmul(
                                g[:],
                                in1_c[:, a, hl, ck * CHUNK : (ck + 1) * CHUNK],
                                in2_c[:, a, hl, ck * CHUNK : ck * CHUNK + BANDW],
                                start=(a == 0),
                                stop=(a == CH - 1),
                            )
                        # band scale 1/C (+ cast to band_dt)
                        nc.scalar.mul(
                            out=band_ts[ck][:, hl, :],
                            in_=g[:],
                            mul=1.0 / C,
                        )

                # --- phase 2: on-chip diagonal extraction (local_scatter:
                # dst[w, hl*DE + (j-w)] = band[w, hl*BANDW + j]) ---
                gat_ts = []
                for ck in range(NCK):
                    gat = gats.tile(
                        [CHUNK, HB, DE], band_dt,
                        name=f"gat{ck}_{_rep}_{ib}", tag=f"gat{ck}",
                    )
                    nc.gpsimd.local_scatter(
                        out_ap=gat[:],
                        data_ap=band_ts[ck][:],
                        idxs_ap=idx_t[:],
                        channels=CHUNK,
                        num_elems=HB * DE,
                        num_idxs=HB * BANDW,
                    )
                    gat_ts.append(gat)

                out_t = outs.tile([D, HB, W], f32)
                for hl in range(HB):
                    po = pso.tile([D, W], band_dt)
                    for ck in range(NCK):
                        nc.tensor.transpose(
                            out=po[:, ck * CHUNK : (ck + 1) * CHUNK],
                            in_=gat_ts[ck][:, hl, 0:D],
                            identity=ident[:],
                        )
                    nc.vector.tensor_copy(out=out_t[:, hl, :], in_=po[:])
                nc.sync.dma_start(out=out_ap[:, h0 : h0 + HB, :], in_=out_t[:])

    nc.compile()
    return nc


_NC_CACHE = None


def run(input1, input2, trace=False, **spmd_kwargs):
    """Run on 8 NeuronCores; returns (out [B,D,H,W] fp32, BassKernelResults)."""
    global _NC_CACHE
    if _NC_CACHE is None:
        _NC_CACHE = _build()
    nc = _NC_CACHE

    input1 = np.ascontiguousarray(np.asarray(input1), dtype=np.float32)
    input2 = np.ascontiguousarray(np.asarray(input2), dtype=np.float32)
    assert input1.shape == (B, C, H, W) and input2.shape == (B, C, H, W)

    didx = make_diag_idx()
    in_maps = [
        {"input1": input1[b], "input2": input2[b], "didx": didx}
        for b in range(B)
    ]
    res = run_bass_kernel_spmd(
        nc, in_maps, core_ids=list(range(B)), trace=trace, **spmd_kwargs
    )
    out = np.stack([res.results[b]["out"] for b in range(B)], axis=0)
    return out, res


def kernel(input1, input2):
    out, _ = run(input1, input2)
    return out


# revision 11
# speedup vs baseline: 4.6204x; 4.6204x over previous
"""Correlation1D Trainium2 Bass kernel.

out[b, d, h, w] = (1/C) * sum_c in1[b, c, h, w] * in2pad[b, c, h, w + d]
  B=8, C=256, H=96, W=192, PAD=40, D=81 displacement channels.

Strategy (data-parallel over batch, 1 sample per NeuronCore):
  For each h row and each w-chunk of 96, compute a Gram band
      G[w, w'] = sum_c in1[c, w] * in2pad[c, w']      (PE matmuls, k=c)
  for w' in [ck*96, ck*96 + 176).  The needed outputs are the 81
  diagonals O[d, w] = G[w, w + d].  Diagonal extraction is a
  per-partition-offset move: partition w needs band columns [w, w+81).
  GPSIMD's local_scatter supports per-partition independent indices
  (dst[p, idx[p,i]] = data[p,i], -1 skips), so a static int16 index
  tile (fed as an extra kernel input) extracts all diagonals on-chip —
  no DRAM scratch round-trip.  HBM traffic is just inputs + output
  (43.7 MB/core).  A PE transpose then turns T[w, d] tiles into
  O[d, w] tiles written out in the final [d, h, w] layout.

  Inputs load as fp32 over HWDGE (keeping descriptor-gen off GPSIMD,
  which local_scatter saturates) and are cast fp32->fp16 on the DVE.
  Matmuls run in fp16 (full PE rate at any moving width, so the rhs is
  just the 176-wide band — no 256-pad needed).  fp16 inputs keep
  ~2^-11 element error; the fp16 band (pre-scaled by 1/C) adds ~5e-4.
"""

import os

import numpy as np

import concourse.bass as bass
import concourse.tile as tile
from concourse import bacc, mybir
from concourse.bass_utils import run_bass_kernel_spmd

# Problem constants (hardcoded per harness contract)
B = 8
C = 256
H = 96
W = 192
PAD = 40
D = 2 * PAD + 1  # 81
W2 = W + 2 * PAD  # 272 padded width
CH = 2  # c is split into CH partition-halves of 128
CP = C // CH  # 128
CHUNK = 96  # w-chunk (Gram output partition dim)
NCK = W // CHUNK  # 2
BANDW = CHUNK + D - 1  # 176  (w' window width per chunk)
DE = D + 1  # 82: even-sized diagonal slot per h row (local_scatter needs %2)

# Tunables (env-overridable for experiments)
HB = int(os.environ.get("CORR_HB", "4"))  # h rows per batch
NB = H // HB
BAND_DT_S = os.environ.get("CORR_BAND_DT", "fp16")  # fp16 | bf16
MM_DT_S = os.environ.get("CORR_MM", "fp16")  # fp16 | bf16
IN_BUFS = int(os.environ.get("CORR_IN_BUFS", "3"))
G_BUFS = int(os.environ.get("CORR_G_BUFS", "4"))

_DT = {
    "fp32": mybir.dt.float32,
    "fp16": mybir.dt.float16,
    "bf16": mybir.dt.bfloat16,
}


def make_diag_idx() -> np.ndarray:
    """Static local_scatter indices: idx[w, hl*BANDW + j] = hl*DE + (j - w)
    when 0 <= j - w < D, else -1 (skipped)."""
    idx = np.full((CHUNK, HB * BANDW), -1, dtype=np.int16)
    w = np.arange(CHUNK)[:, None]
    j = np.arange(BANDW)[None, :]
    d = j - w  # [CHUNK, BANDW]
    valid = (d >= 0) & (d < D)
    for hl in range(HB):
        blk = np.where(valid, hl * DE + d, -1).astype(np.int16)
        idx[:, hl * BANDW : (hl + 1) * BANDW] = blk
    return idx


def _build(reps=1):
    band_dt = _DT[BAND_DT_S]
    mm_dt = _DT[MM_DT_S]
    f32 = mybir.dt.float32
    i16 = mybir.dt.int16

    nc = bacc.Bacc("TRN2")

    in1 = nc.dram_tensor("input1", [C, H, W], f32, kind="ExternalInput")
    in2 = nc.dram_tensor("input2", [C, H, W], f32, kind="ExternalInput")
    didx = nc.dram_tensor("didx", [CHUNK, HB * BANDW], i16, kind="ExternalInput")
    out = nc.dram_tensor("out", [D, H, W], f32, kind="ExternalOutput")

    # [c, h, w] -> [p, a, h*w] so each input load is one 3-dim DMA
    in1_r = in1.ap().rearrange("(a p) h w -> p a (h w)", p=CP)
    in2_r = in2.ap().rearrange("(a p) h w -> p a (h w)", p=CP)
    out_ap = out.ap()

    with tile.TileContext(nc) as tc:
        with (
            tc.tile_pool(name="singles", bufs=1) as singles,
            tc.tile_pool(name="loads", bufs=IN_BUFS) as loads,
            tc.tile_pool(name="casts", bufs=2) as casts,
            tc.tile_pool(name="bands", bufs=2) as bands,
            tc.tile_pool(name="gats", bufs=2) as gats,
            tc.tile_pool(name="outs", bufs=2) as outs,
            tc.tile_pool(name="psg", bufs=G_BUFS, space="PSUM") as psg,
            tc.tile_pool(name="pso", bufs=2, space="PSUM") as pso,
        ):
            # identity for PE transposes
            ident = singles.tile([CHUNK, CHUNK], band_dt)
            from concourse.masks import make_identity

            make_identity(nc, ident[:])

            # static per-partition diagonal indices, loaded once
            idx_t = singles.tile([CHUNK, HB * BANDW], i16)
            nc.sync.dma_start(out=idx_t[:], in_=didx.ap())

            for _rep in range(reps):
              for ib in range(NB):
                h0 = ib * HB

                # split the two big loads across two HWDGE queues (scalar +
                # sync) so DMA isn't bottlenecked on one ring
                in1_t = loads.tile([CP, CH, HB * W], f32)
                nc.scalar.dma_start(
                    out=in1_t[:],
                    in_=in1_r[:, :, h0 * W : (h0 + HB) * W],
                )
                in2_t = loads.tile([CP, CH, HB * W], f32)
                nc.sync.dma_start(
                    out=in2_t[:],
                    in_=in2_r[:, :, h0 * W : (h0 + HB) * W],
                )

                # fp32 -> mm_dt casts on DVE; in2 goes into the interior of
                # a zero-padded 272-wide tile (band windows reach the pads)
                in1_c = casts.tile([CP, CH, HB, W], mm_dt)
                nc.vector.tensor_copy(
                    out=in1_c[:].rearrange("p a h w -> p a (h w)"), in_=in1_t[:]
                )
                in2_c = casts.tile([CP, CH, HB, W2], mm_dt)
                nc.vector.memset(in2_c[:, :, :, 0:PAD], 0.0)
                nc.vector.memset(in2_c[:, :, :, PAD + W : W2], 0.0)
                nc.vector.tensor_copy(
                    out=in2_c[:, :, :, PAD : PAD + W],
                    in_=in2_t[:].rearrange("p a (h w) -> p a h w", h=HB),
                )

                band_ts = [
                    bands.tile(
                        [CHUNK, HB, BANDW], band_dt,
                        name=f"band{ck}_{_rep}_{ib}", tag=f"band{ck}",
                    )
                    for ck in range(NCK)
                ]

                for hl in range(HB):
                    for ck in range(NCK):
                        g = psg.tile([CHUNK, BANDW], f32)
                        for a in range(CH):
                            nc.tensor.matmul(
                                g[:],
                                in1_c[:, a, hl, ck * CHUNK : (ck + 1) * CHUNK],
                                in2_c[:, a, hl, ck * CHUNK : ck * CHUNK + BANDW],
                                start=(a == 0),
                                stop=(a == CH - 1),
                            )
                        # band scale 1/C (+ cast to band_dt)
                        nc.scalar.mul(
                            out=band_ts[ck][:, hl, :],
                            in_=g[:],
                            mul=1.0 / C,
                        )

                # --- phase 2: on-chip diagonal extraction (local_scatter:
                # dst[w, hl*DE + (j-w)] = band[w, hl*BANDW + j]) ---
                gat_ts = []
                for ck in range(NCK):
                    gat = gats.tile(
                        [CHUNK, HB, DE], band_dt,
                        name=f"gat{ck}_{_rep}_{ib}", tag=f"gat{ck}",
                    )
                    nc.gpsimd.local_scatter(
                        out_ap=gat[:],
                        data_ap=band_ts[ck][:],
                        idxs_ap=idx_t[:],
                        channels=CHUNK,
                        num_elems=HB * DE,
                        num_idxs=HB * BANDW,
                    )
                    gat_ts.append(gat)

                out_t = outs.tile([D, HB, W], f32)
                for hl in range(HB):
                    po = pso.tile([D, W], band_dt)
                    for ck in range(NCK):
                        nc.tensor.transpose(
                            out=po[:, ck * CHUNK : (ck + 1) * CHUNK],
                            in_=gat_ts[ck][:, hl, 0:D],
                            identity=ident[:],
                        )
                    nc.vector.tensor_copy(out=out_t[:, hl, :], in_=po[:])
                nc.sync.dma_start(out=out_ap[:, h0 : h0 + HB, :], in_=out_t[:])

    nc.compile()
    return nc


_NC_CACHE = None


def run(input1, input2, trace=False, **spmd_kwargs):
    """Run on 8 NeuronCores; returns (out [B,D,H,W] fp32, BassKernelResults)."""
    global _NC_CACHE
    if _NC_CACHE is None:
        _NC_CACHE = _build()
    nc = _NC_CACHE

    input1 = np.ascontiguousarray(np.asarray(input1), dtype=np.float32)
    input2 = np.ascontiguousarray(np.asarray(input2), dtype=np.float32)
    assert input1.shape == (B, C, H, W) and input2.shape == (B, C, H, W)

    didx = make_diag_idx()
    in_maps = [
        {"input1": input1[b], "input2": input2[b], "didx": didx}
        for b in range(B)
    ]
    res = run_bass_kernel_spmd(
        nc, in_maps, core_ids=list(range(B)), trace=trace, **spmd_kwargs
    )
    out = np.stack([res.results[b]["out"] for b in range(B)], axis=0)
    return out, res


def kernel(input1, input2):
    out, _ = run(input1, input2)
    return out
